# revision 2
# baseline (speedup 1.0000x reference)
"""Trainium2 Bass kernel for nn_MultiHeadClusterAttention (sparse clustered attention).

Sharding: sequence-parallel over n across 8 NeuronCores; centroids replicated;
kmeans centroid sums/counts AllReduced each iteration.

Numerics (validated on CPU vs the f32 reference, final absmax-rel ~2.3e-3):
 - x, centroids held as fp16 hi/lo pairs (exact products in PE, ~21-bit total)
 - kmeans distances: 3-term fp16 (hh + hl + lh), full-rate matmuls
 - kmeans scatter: transposed one-hot scatter, 2-term fp16 (oh @ xhi + oh @ xlo)
   accumulated in f32 PSUM -> sums land directly in C^T layout (no per-iter
   centroid transposes)
 - attention: 1-term fp16 projections/energies, bf16 exp (range) x bf16 V,
   1/Z via DVE reciprocal + one Newton step (no Ln/Exp activation-table thrash)
 - output: fp16 (bias folded on device), host upcasts to f32
Host sends: x as fp16 hi/lo pair (sharded), W^T as fp16 (replicated),
initial C^T as fp16 pair + c^2 row, biases.
"""
import numpy as np

import concourse.bacc as bacc
import concourse.mybir as mybir
import concourse.tile as tile

NCORES = 8
N, E, NH = 32768, 512, 8
KC = N // 100            # 327
ITERS = 10
NL = N // NCORES         # 4096
P = 128
NCH = NL // P            # 32
NS = NL // 512           # 8 point-blocks of 512
NB = [(0, 128), (128, 128), (256, 71)]
INVSQRT_E = 1.0 / float(np.sqrt(np.float32(E)))

f32 = mybir.dt.float32
f16 = mybir.dt.float16
bf16 = mybir.dt.bfloat16
AF = mybir.ActivationFunctionType
ALU = mybir.AluOpType
AX = mybir.AxisListType

# jnp.linspace(0, N-1, KC).astype(int32) on CPU jax (harness reference backend)
INIT_IDX = np.array([0, 100, 201, 301, 402, 502, 603, 703, 804, 904, 1005, 1105, 1206, 1306, 1407, 1507, 1608, 1708, 1809, 1909, 2010, 2110, 2211, 2311, 2412, 2512, 2613, 2713, 2814, 2914, 3015, 3115, 3216, 3316, 3417, 3517, 3618, 3718, 3819, 3919, 4020, 4121, 4221, 4322, 4422, 4523, 4623, 4724, 4824, 4925, 5025, 5126, 5226, 5327, 5427, 5528, 5628, 5729, 5829, 5930, 6030, 6131, 6231, 6332, 6432, 6533, 6633, 6734, 6834, 6935, 7035, 7136, 7236, 7337, 7437, 7538, 7638, 7739, 7839, 7940, 8040, 8141, 8242, 8342, 8443, 8543, 8644, 8744, 8845, 8945, 9046, 9146, 9247, 9347, 9448, 9548, 9649, 9749, 9850, 9950, 10051, 10151, 10252, 10352, 10453, 10553, 10654, 10754, 10855, 10955, 11056, 11156, 11257, 11357, 11458, 11558, 11659, 11759, 11860, 11960, 12061, 12161, 12262, 12363, 12463, 12564, 12664, 12765, 12865, 12966, 13066, 13167, 13267, 13368, 13468, 13569, 13669, 13770, 13870, 13971, 14071, 14172, 14272, 14373, 14473, 14574, 14674, 14775, 14875, 14976, 15076, 15177, 15277, 15378, 15478, 15579, 15679, 15780, 15880, 15981, 16081, 16182, 16282, 16383, 16484, 16584, 16685, 16785, 16886, 16986, 17087, 17187, 17288, 17388, 17489, 17589, 17690, 17790, 17891, 17991, 18092, 18192, 18293, 18393, 18494, 18594, 18695, 18795, 18896, 18996, 19097, 19197, 19298, 19398, 19499, 19599, 19700, 19800, 19901, 20001, 20102, 20202, 20303, 20403, 20504, 20605, 20705, 20806, 20906, 21007, 21107, 21208, 21308, 21409, 21509, 21610, 21710, 21811, 21911, 22012, 22112, 22213, 22313, 22414, 22514, 22615, 22715, 22816, 22916, 23017, 23117, 23218, 23318, 23419, 23519, 23620, 23720, 23821, 23921, 24022, 24122, 24223, 24323, 24424, 24524, 24625, 24726, 24826, 24927, 25027, 25128, 25228, 25329, 25429, 25530, 25630, 25731, 25831, 25932, 26032, 26133, 26233, 26334, 26434, 26535, 26635, 26736, 26836, 26937, 27037, 27138, 27238, 27339, 27439, 27540, 27640, 27741, 27841, 27942, 28042, 28143, 28243, 28344, 28444, 28545, 28645, 28746, 28847, 28947, 29048, 29148, 29249, 29349, 29450, 29550, 29651, 29751, 29852, 29952, 30053, 30153, 30254, 30354, 30455, 30555, 30656, 30756, 30857, 30957, 31058, 31158, 31259, 31359, 31460, 31560, 31661, 31761, 31862, 31962, 32063, 32163, 32264, 32364, 32465, 32565, 32666, 32767], dtype=np.int32)


def _build(n_iters=ITERS, warmar=True, attn_recip="approx", km_recip="approx"):
    nc = bacc.Bacc("TRN2", target_bir_lowering=False, debug=False, num_devices=NCORES)
    xhd = nc.dram_tensor("xh", [NL, E], f16, kind="ExternalInput")
    xld = nc.dram_tensor("xl", [NL, E], f16, kind="ExternalInput")
    wd = {w: nc.dram_tensor(w, [E, E], f16, kind="ExternalInput")
          for w in ("wqt", "wkt", "wvt", "wot")}
    cthd = nc.dram_tensor("c0th", [P, 4 * KC], f16, kind="ExternalInput")
    ctld = nc.dram_tensor("c0tl", [P, 4 * KC], f16, kind="ExternalInput")
    c20d = nc.dram_tensor("c20", [1, KC], f32, kind="ExternalInput")
    bqd = nc.dram_tensor("bq", [E], f32, kind="ExternalInput")
    bkd = nc.dram_tensor("bk", [E], f32, kind="ExternalInput")
    outbd = nc.dram_tensor("outb", [1, E], f32, kind="ExternalInput")
    outd = nc.dram_tensor("out", [NL, E], f16, kind="ExternalOutput")

    # Qproj sliver assignment: 32 (r, s) groups over iterations
    qgroups = [[] for _ in range(n_iters)]
    for g in range(32):
        qgroups[min(g // 3, n_iters - 1)].append((g // 8, g % 8))

    with tile.TileContext(nc) as tc:
        with (
            tc.tile_pool(name="sbp", bufs=1) as sbp,
            tc.tile_pool(name="dram", bufs=1, space="DRAM") as dram,
        ):
            # ---------- persistent SBUF (whole kernel) ----------
            qsb = sbp.tile([P, 4 * NL], f16, tag="qsb")          # Q: slice r at cols [NL*r:]
            cth = sbp.tile([P, 4 * KC], f16, tag="cth")          # C^T hi: e-slice r at [KC*r:]
            ctl = sbp.tile([P, 4 * KC], f16, tag="ctl")
            c2t = sbp.tile([P, KC], f32, tag="c2t")
            ones16 = sbp.tile([P, 1], f16, tag="ones16")
            ones32 = sbp.tile([P, 1], f32, tag="ones32")
            onesrow = sbp.tile([1, KC], f32, tag="onesrow")
            bq_c = sbp.tile([P, 4], f32, tag="bqc")
            bk_c = sbp.tile([P, 4], f32, tag="bkc")
            nbias = sbp.tile([P, 1], f32, tag="nbias")
            arin = dram.tile([129, 4 * KC], f32, tag="arin")
            arout = dram.tile([129, 4 * KC], f32, tag="arout")

            nc.gpsimd.memset(ones16[:], 1.0)
            nc.gpsimd.memset(ones32[:], 1.0)
            nc.gpsimd.memset(onesrow[:], 1.0)
            nc.gpsimd.memset(nbias[:], -39.0)
            for r in range(4):
                nc.sync.dma_start(bq_c[:, r:r + 1], bqd[P * r:P * (r + 1)])
                nc.sync.dma_start(bk_c[:, r:r + 1], bkd[P * r:P * (r + 1)])
            nc.sync.dma_start(cth[:], cthd[:])
            nc.sync.dma_start(ctl[:], ctld[:])
            # warm-up AllReduces: absorb collective cold-start + core skew
            # behind the prep phase (first real AR otherwise costs ~160us).
            # The second warm-up uses the real arin/arout buffers at full size
            # (the cold cost is partly message-size specific).
            war_in = dram.tile([1, 8], f32, tag="warin")
            war_out = dram.tile([1, 8], f32, tag="warout")
            wz = sbp.tile([1, 8], f32, tag="wz")
            nc.gpsimd.memset(wz[:], 0.0)
            nc.sync.dma_start(war_in[:], wz[:])
            if warmar:
                nc.gpsimd.collective_compute(
                    "AllReduce", ALU.add, replica_groups=[list(range(NCORES))],
                    ins=[war_in.opt()], outs=[war_out.opt()],
                )

            with (
                tc.tile_pool(name="sbk", bufs=1) as sbk,
                tc.tile_pool(name="d2p", bufs=3) as d2p,
                tc.tile_pool(name="upd", bufs=1) as upd,
            ):
                # x fp16 pair, point-major [128 pts, E] per chunk
                xph = sbk.tile([P, NCH * E], f16, tag="xph")
                xpl = sbk.tile([P, NCH * E], f16, tag="xpl")
                # x fp16 pair, E-major: slice r cols [NL*r : NL*r+NL]
                xth = sbk.tile([P, 4 * NL], f16, tag="xth")
                xtl = sbk.tile([P, 4 * NL], f16, tag="xtl")
                wqt = sbk.tile([P, 4 * E], f16, tag="wqt")       # Wq^T: e-slice k at [E*k:]
                for k in range(4):
                    nc.sync.dma_start(wqt[:, E * k:E * (k + 1)], wd["wqt"][P * k:P * (k + 1), :])

                with (
                    tc.tile_pool(name="sbprep", bufs=1) as sbprep,
                    tc.tile_pool(name="psT", bufs=4, space="PSUM") as psT,
                ):
                    it32 = sbprep.tile([P, P], mybir.dt.int32, tag="it32")
                    ident = sbprep.tile([P, P], f16, tag="ident")
                    nc.gpsimd.iota(it32[:], [[1, P]], base=0, channel_multiplier=-1)
                    nc.vector.tensor_scalar(ident[:], it32[:], 0, None, ALU.is_equal)
                    c2r0 = sbprep.tile([1, KC], f32, tag="c2r0")
                    nc.sync.dma_start(c2r0[:], c20d[:])
                    nc.gpsimd.partition_broadcast(c2t[:], c2r0[:])
                    # zero the junk tail of arin row 128 (cnt row occupies cols 0:KC)
                    for ch in range(NCH):
                        nc.sync.dma_start(xph[:, E * ch:E * (ch + 1)],
                                          xhd[P * ch:P * (ch + 1), :])
                        nc.sync.dma_start(xpl[:, E * ch:E * (ch + 1)],
                                          xld[P * ch:P * (ch + 1), :])
                        for r in range(4):
                            tp = psT.tile([P, P], f16, tag="tr", name="tp")
                            nc.tensor.transpose(tp[:], xph[:, E * ch + P * r:E * ch + P * (r + 1)],
                                                ident[:])
                            nc.scalar.activation(
                                xth[:, NL * r + P * ch:NL * r + P * (ch + 1)], tp[:], AF.Copy)
                            tpl = psT.tile([P, P], f16, tag="tr", name="tpl")
                            nc.tensor.transpose(tpl[:], xpl[:, E * ch + P * r:E * ch + P * (r + 1)],
                                                ident[:])
                            nc.scalar.activation(
                                xtl[:, NL * r + P * ch:NL * r + P * (ch + 1)], tpl[:], AF.Copy)
                    # full-size warm-up AllReduce, emitted after the x loads so
                    # it overlaps the transpose phase (absorbs the size-specific
                    # part of the collective cold cost)
                    zbig = sbprep.tile([P, 4 * KC], f32, tag="zbig")
                    nc.gpsimd.memset(zbig[:], 0.0)
                    nc.sync.dma_start(arin[0:128, :], zbig[:])
                    nc.sync.dma_start(arin[128:129, :], zbig[:1, :])
                    if warmar:
                        nc.gpsimd.collective_compute(
                            "AllReduce", ALU.add,
                            replica_groups=[list(range(NCORES))],
                            ins=[arin.opt()], outs=[arout.opt()],
                        )

                # ======== kmeans ========
                kmeans_pools = tc.tile_pool(name="psK", bufs=3, space="PSUM")
                psK = kmeans_pools.__enter__()
                psS_cm = tc.tile_pool(name="psS", bufs=1, space="PSUM")
                psS = psS_cm.__enter__()
                for it in range(n_iters):
                    sps = [psS.tile([P, 512], f32, tag=f"s{r}", name=f"sps{r}")
                           for r in range(4)]
                    cps = psS.tile([P, 512], f32, tag="cnt", name="cps")

                    def emit_scatter(ch, oh):
                        for r in range(4):
                            nc.tensor.matmul(sps[r][:, :KC],
                                             xph[:, E * ch + P * r:E * ch + P * (r + 1)],
                                             oh[:], start=(ch == 0), stop=False)
                            nc.tensor.matmul(sps[r][:, :KC],
                                             xpl[:, E * ch + P * r:E * ch + P * (r + 1)],
                                             oh[:], start=False, stop=(ch == NCH - 1))
                        nc.tensor.matmul(cps[:1, :KC], ones16[:], oh[:],
                                         start=(ch == 0), stop=(ch == NCH - 1))

                    # software pipeline: the scatter of chunk ch-1 is emitted
                    # between chunk ch's distance matmuls and its DVE argmin,
                    # so the PE never waits on the DVE chain.
                    prev_oh = None
                    for ch in range(NCH):
                        dps = psK.tile([P, 512], f32, tag="dps", name="dps")
                        for r in range(4):
                            xh_sl = xth[:, NL * r + P * ch:NL * r + P * (ch + 1)]
                            xl_sl = xtl[:, NL * r + P * ch:NL * r + P * (ch + 1)]
                            ch_sl = cth[:, KC * r:KC * (r + 1)]
                            cl_sl = ctl[:, KC * r:KC * (r + 1)]
                            nc.tensor.matmul(dps[:, :KC], xh_sl, ch_sl,
                                             start=(r == 0), stop=False)
                            nc.tensor.matmul(dps[:, :KC], xh_sl, cl_sl,
                                             start=False, stop=False)
                            nc.tensor.matmul(dps[:, :KC], xl_sl, ch_sl,
                                             start=False, stop=(r == 3))
                        if prev_oh is not None:
                            emit_scatter(ch - 1, prev_oh)
                        d2 = d2p.tile([P, KC], f32, tag="d2")
                        nc.vector.scalar_tensor_tensor(d2[:], dps[:, :KC], -2.0, c2t[:],
                                                       ALU.mult, ALU.add)
                        mn = d2p.tile([P, 1], f32, tag="mn")
                        nc.vector.tensor_reduce(mn[:], d2[:], AX.X, ALU.min)
                        oh = d2p.tile([P, KC], f16, tag="oh")
                        nc.vector.tensor_scalar(oh[:], d2[:], mn[:], None, ALU.is_le)
                        prev_oh = oh
                    emit_scatter(NCH - 1, prev_oh)
                    # stage + AllReduce (stage shares the buffer with `red`)
                    stage = upd.tile([P, 4 * KC], f32, tag="red", name="stage")
                    cntrow = upd.tile([1, KC], f32, tag="cntrow", name="cntrow")
                    for r in range(4):
                        nc.scalar.activation(stage[:, KC * r:KC * (r + 1)],
                                             sps[r][:, :KC], AF.Copy)
                    nc.scalar.activation(cntrow[:], cps[:1, :KC], AF.Copy)
                    nc.sync.dma_start(arin[0:128, :], stage[:])
                    nc.sync.dma_start(arin[128:129, 0:KC], cntrow[:])
                    nc.gpsimd.collective_compute(
                        "AllReduce", ALU.add, replica_groups=[list(range(NCORES))],
                        ins=[arin.opt()], outs=[arout.opt()],
                    )
                    # Qproj slivers overlap the AllReduce
                    for (r, s) in qgroups[it]:
                        qps = psK.tile([P, 512], f32, tag="dps", name="qps")
                        for k in range(4):
                            nc.tensor.matmul(
                                qps[:], wqt[:, E * k + P * r:E * k + P * (r + 1)],
                                xth[:, NL * k + 512 * s:NL * k + 512 * (s + 1)],
                                start=(k == 0), stop=(k == 3))
                        nc.vector.tensor_scalar(qsb[:, NL * r + 512 * s:NL * r + 512 * (s + 1)],
                                                qps[:], bq_c[:, r:r + 1], None, ALU.add)
                    # read back + update centroids
                    red = upd.tile([P, 4 * KC], f32, tag="red", name="red")
                    nc.sync.dma_start(red[:], arout[0:128, :])
                    cr = upd.tile([1, KC], f32, tag="cr", name="cr")
                    nc.sync.dma_start(cr[:], arout[128:129, 0:KC])
                    ra = upd.tile([1, KC], f32, tag="ra", name="ra")
                    rb = upd.tile([1, KC], f32, tag="rb", name="rb")
                    rc = upd.tile([1, KC], f32, tag="rc", name="rc")
                    rd = upd.tile([1, KC], f32, tag="rd", name="rd")
                    # ra = max(cnt,1); rb = 1/ra (approx reciprocal + one Newton step)
                    nc.vector.tensor_scalar(ra[:], cr[:], 1.0, None, ALU.max)
                    nc.vector.reciprocal(rb[:], ra[:])
                    nc.vector.tensor_tensor(rc[:], ra[:], rb[:], ALU.mult)
                    nc.vector.tensor_tensor(rd[:], onesrow[:], rc[:], ALU.subtract)
                    nc.vector.tensor_tensor(rc[:], rb[:], rd[:], ALU.mult)
                    nc.vector.tensor_tensor(rb[:], rb[:], rc[:], ALU.add)
                    nc.vector.tensor_scalar(ra[:], cr[:], 0.0, None, ALU.is_gt)
                    nc.vector.tensor_tensor(rb[:], rb[:], ra[:], ALU.mult)
                    nc.vector.tensor_scalar(ra[:], cr[:], 0.0, None, ALU.is_le)
                    fb = upd.tile([P, KC], f32, tag="fb", name="fb")
                    nc.gpsimd.partition_broadcast(fb[:], rb[:])
                    nb_ = upd.tile([P, KC], f32, tag="nb", name="nb_")
                    nc.gpsimd.partition_broadcast(nb_[:], ra[:])
                    c2ps = psK.tile([P, 512], f32, tag="dps", name="c2ps")
                    for r in range(4):
                        u1 = upd.tile([P, KC], f32, tag="u1", name="u1")
                        u2 = upd.tile([P, KC], f32, tag="u2", name="u2")
                        u3 = upd.tile([P, KC], f32, tag="u3", name="u3")
                        nc.vector.tensor_tensor(u1[:], cth[:, KC * r:KC * (r + 1)],
                                                ctl[:, KC * r:KC * (r + 1)], ALU.add)
                        nc.vector.tensor_tensor(u2[:], red[:, KC * r:KC * (r + 1)], fb[:],
                                                ALU.mult)
                        nc.vector.tensor_tensor(u3[:], u1[:], nb_[:], ALU.mult)
                        tmp = upd.tile([P, KC], f32, tag="u1", name="tmp")
                        nc.vector.tensor_tensor(tmp[:], u2[:], u3[:], ALU.add)
                        nc.vector.tensor_copy(cth[:, KC * r:KC * (r + 1)], tmp[:])
                        nc.vector.tensor_tensor(ctl[:, KC * r:KC * (r + 1)], tmp[:],
                                                cth[:, KC * r:KC * (r + 1)], ALU.subtract)
                        sq = upd.tile([P, KC], f32, tag="u2", name="sq")
                        nc.vector.tensor_tensor(sq[:], tmp[:], tmp[:], ALU.mult)
                        nc.tensor.matmul(c2ps[:1, :KC], ones32[:], sq[:],
                                         start=(r == 0), stop=(r == 3))
                    c2r = upd.tile([1, KC], f32, tag="cr", name="c2r")
                    nc.scalar.activation(c2r[:], c2ps[:1, :KC], AF.Copy)
                    nc.gpsimd.partition_broadcast(c2t[:], c2r[:])
                psS_cm.__exit__(None, None, None)
                kmeans_pools.__exit__(None, None, None)

            # ======== attention ========
            with (
                tc.tile_pool(name="sba", bufs=1) as sba,
                tc.tile_pool(name="expp", bufs=14) as expp,
                tc.tile_pool(name="alp", bufs=4) as alp,
                tc.tile_pool(name="oatp", bufs=2) as oatp,
                tc.tile_pool(name="outp", bufs=3) as outp,
                tc.tile_pool(name="psA", bufs=2, space="PSUM") as psA,
                tc.tile_pool(name="psU", bufs=2, space="PSUM") as psU,
                tc.tile_pool(name="psF", bufs=2, space="PSUM") as psF,
            ):
                wsb = {}
                for w in ("wkt", "wvt", "wot"):
                    wsb[w] = sba.tile([P, 4 * E], f16, tag=w, name=w)
                    for k in range(4):
                        nc.sync.dma_start(wsb[w][:, E * k:E * (k + 1)],
                                          wd[w][P * k:P * (k + 1), :])
                outb_bc = sba.tile([P, E], f32, tag="outbbc")
                obrow = sba.tile([1, E], f32, tag="obrow")
                nc.sync.dma_start(obrow[:], outbd[:])
                nc.gpsimd.partition_broadcast(outb_bc[:], obrow[:])
                # KT = Wk @ C^T + bk   [128 eo-slice r, KC]
                kt = sba.tile([P, 4 * KC], f16, tag="kt")
                for r in range(4):
                    kps = psA.tile([P, 512], f32, tag="eA", name="kps")
                    for k in range(4):
                        nc.tensor.matmul(kps[:, :KC],
                                         wsb["wkt"][:, E * k + P * r:E * k + P * (r + 1)],
                                         cth[:, KC * k:KC * (k + 1)],
                                         start=(k == 0), stop=(k == 3))
                    nc.vector.tensor_scalar(kt[:, KC * r:KC * (r + 1)], kps[:, :KC],
                                            bk_c[:, r:r + 1], None, ALU.add)
                # Vaug bf16 (x 1/sqrt(E); ones col per head)
                onesb = sba.tile([P, 1], bf16, tag="onesb")
                nc.vector.tensor_copy(onesb[:], ones32[:])
                vaug = []
                for b, (b0, nb) in enumerate(NB):
                    va = sba.tile([P, 65 * NH], bf16, tag=f"vaug{b}", name=f"vaug{b}")
                    vaug.append(va)
                    vps = psU.tile([P, 512], f32, tag="u", name="vps")
                    for k in range(4):
                        nc.tensor.matmul(vps[:nb, :],
                                         cth[:, KC * k + b0:KC * k + b0 + nb],
                                         wsb["wvt"][:, E * k:E * (k + 1)],
                                         start=(k == 0), stop=(k == 3))
                    for h in range(NH):
                        nc.scalar.activation(va[:nb, 65 * h:65 * h + 64],
                                             vps[:nb, 64 * h:64 * (h + 1)],
                                             AF.Copy, scale=INVSQRT_E)
                        nc.vector.tensor_copy(va[:nb, 65 * h + 64:65 * (h + 1)], onesb[:nb])

                def emit_energy(s, i):
                    q0 = qsb[0:64, NL * i + 512 * s:NL * i + 512 * (s + 1)]
                    q1 = qsb[64:128, NL * i + 512 * s:NL * i + 512 * (s + 1)]
                    ex = {0: [], 1: []}
                    for b, (b0, nb) in enumerate(NB):
                        eA = psA.tile([P, 512], f32, tag="eA", name="eA")
                        nc.tensor.matmul(eA[:nb, :], kt[0:64, KC * i + b0:KC * i + b0 + nb],
                                         q0, start=True, stop=True)
                        eB = psA.tile([P, 512], f32, tag="eB", name="eB")
                        nc.tensor.matmul(eB[:nb, :], kt[64:128, KC * i + b0:KC * i + b0 + nb],
                                         q1, start=True, stop=True)
                        xA = expp.tile([P, 512], bf16, tag="ex", name="xA")
                        nc.scalar.activation(xA[:nb, :], eA[:nb, :], AF.Exp, bias=nbias[:nb])
                        xB = expp.tile([P, 512], bf16, tag="ex", name="xB")
                        nc.scalar.activation(xB[:nb, :], eB[:nb, :], AF.Exp, bias=nbias[:nb])
                        ex[0].append(xA)
                        ex[1].append(xB)
                    return ex

                def emit_attv(s, i, ex, oats):
                    oat = oatp.tile([P, 512], f16, tag=f"oat{i}", name=f"oat{i}")
                    oats.append(oat)
                    for hl in range(2):
                        h = 2 * i + hl
                        ups = psU.tile([65, 512], f32, tag="u", name="ups")
                        for b, (b0, nb) in enumerate(NB):
                            nc.tensor.matmul(ups[:], vaug[b][:nb, 65 * h:65 * (h + 1)],
                                             ex[hl][b][:nb, :], start=(b == 0), stop=(b == 2))
                        # alpha = 1/Z (~18-bit approx). Stage Z to SBUF with a
                        # +1e-30 floor (scalar Copy bias): approx recip needs a
                        # normal fp32 SBUF input.
                        zrow = alp.tile([1, 512], f32, tag="zrow")
                        nc.scalar.activation(zrow[:], ups[64:65, :], AF.Copy,
                                             bias=1e-30)
                        arow = alp.tile([1, 512], f32, tag="arow")
                        if attn_recip == "approx":
                            nc.vector.reciprocal_approx_fast(arow[:], zrow[:])
                        else:
                            nc.vector.reciprocal(arow[:], zrow[:])
                        ab = alp.tile([64, 512], f32, tag="ab")
                        nc.gpsimd.partition_broadcast(ab[:], arow[:])
                        nc.vector.tensor_tensor(oat[64 * hl:64 * (hl + 1), :],
                                                ups[0:64, :], ab[:], ALU.mult)

                for s in range(NS):
                    # software pipeline: energies of block i+1 are emitted before
                    # the attV/alpha of block i, keeping the PE dense (HAM warm)
                    oats = []
                    exs = {}
                    exs[0] = emit_energy(s, 0)
                    for i in range(1, 4):
                        exs[i] = emit_energy(s, i)
                        emit_attv(s, i - 1, exs.pop(i - 1), oats)
                    emit_attv(s, 3, exs.pop(3), oats)
                    for m in range(4):
                        fps = psF.tile([P, 512], f32, tag="f", name="fps")
                        for r in range(4):
                            nc.tensor.matmul(fps[:], oats[r][:, P * m:P * (m + 1)],
                                             wsb["wot"][:, E * r:E * (r + 1)],
                                             start=(r == 0), stop=(r == 3))
                        ot = outp.tile([P, 512], f16, tag="ot")
                        nc.vector.tensor_tensor(ot[:], fps[:], outb_bc[:], ALU.add)
                        nc.sync.dma_start(outd[512 * s + P * m:512 * s + P * (m + 1), :], ot[:])

    nc.compile()
    return nc


# ---------------- host side ----------------

_CACHE = {}


def _get_runner():
    if "runner" in _CACHE:
        return _CACHE["runner"]
    import jax
    from jax.sharding import Mesh, PartitionSpec
    from jax.experimental.shard_map import shard_map
    from concourse.bass2jax import (
        _bass_exec_p, install_neuronx_cc_hook, partition_id_tensor)

    nc = _build()
    install_neuronx_cc_hook()
    partition_name = nc.partition_id_tensor.name if nc.partition_id_tensor else None
    in_names, out_names, out_avals = [], [], []
    for alloc in nc.m.functions[0].allocations:
        if not isinstance(alloc, mybir.MemoryLocationSet):
            continue
        name = alloc.memorylocations[0].name
        if alloc.kind == "ExternalInput":
            if name != partition_name:
                in_names.append(name)
        elif alloc.kind == "ExternalOutput":
            out_names.append(name)
            out_avals.append(jax.core.ShapedArray(tuple(alloc.tensor_shape),
                                                  mybir.dt.np(alloc.dtype)))
    n_params = len(in_names)
    n_outs = len(out_avals)
    all_in = list(in_names) + list(out_names)
    if partition_name is not None:
        all_in.append(partition_name)
    donate = tuple(range(n_params, n_params + n_outs))

    def _body(*args):
        operands = list(args)
        if partition_name is not None:
            operands.append(partition_id_tensor())
        return tuple(_bass_exec_p.bind(
            *operands, out_avals=tuple(out_avals), in_names=tuple(all_in),
            out_names=tuple(out_names), lowering_input_output_aliases=(),
            sim_require_finite=True, sim_require_nnan=True, nc=nc))

    devices = jax.devices()[:NCORES]
    mesh = Mesh(np.asarray(devices), ("core",))
    in_specs = (PartitionSpec("core"),) * (n_params + n_outs)
    out_specs = (PartitionSpec("core"),) * len(out_names)
    fn = jax.jit(shard_map(_body, mesh=mesh, in_specs=in_specs,
                           out_specs=out_specs, check_rep=False),
                 donate_argnums=donate, keep_unused=True)
    import jax.numpy as jnp
    from jax.sharding import NamedSharding
    shard = NamedSharding(mesh, PartitionSpec("core"))
    mkzeros = jax.jit(
        lambda: tuple(jnp.zeros((NCORES * a.shape[0], *a.shape[1:]), a.dtype)
                      for a in out_avals),
        out_shardings=tuple(shard for _ in out_avals))
    _CACHE["runner"] = (fn, in_names, out_names, mkzeros, nc)
    return _CACHE["runner"]


def kernel(x, A=None, Wq=None, bq=None, Wk=None, bk=None, Wv=None, bv=None,
           Wo=None, bo=None, **kw):
    x = np.asarray(x, np.float32)
    Wq = np.asarray(Wq, np.float32); Wk = np.asarray(Wk, np.float32)
    Wv = np.asarray(Wv, np.float32); Wo = np.asarray(Wo, np.float32)
    bq = np.asarray(bq, np.float32); bk = np.asarray(bk, np.float32)
    bv = np.asarray(bv, np.float32); bo = np.asarray(bo, np.float32)
    b, n, e = x.shape
    assert (b, n, e) == (1, N, E)
    x0 = x[0]
    xh = x0.astype(np.float16)
    xl = (x0 - xh.astype(np.float32)).astype(np.float16)
    c0 = x0[INIT_IDX].astype(np.float32)
    c0t = np.ascontiguousarray(c0.T)                      # [E, KC]
    c0t_blocks = np.concatenate([c0t[P * r:P * (r + 1)] for r in range(4)], axis=1)
    c0th = c0t_blocks.astype(np.float16)
    c0tl = (c0t_blocks - c0th.astype(np.float32)).astype(np.float16)
    ceff = c0th.astype(np.float32) + c0tl.astype(np.float32)
    c20 = np.zeros((1, KC), np.float32)
    for r in range(4):
        c20[0] += np.sum(ceff[:, KC * r:KC * (r + 1)] ** 2, axis=0)
    outb = ((bv @ Wo.T) * np.float32(INVSQRT_E) + bo).reshape(1, E).astype(np.float32)

    fn, in_names, out_names, mkzeros, _ = _get_runner()
    rep = {
        "wqt": np.ascontiguousarray(Wq.T).astype(np.float16),
        "wkt": np.ascontiguousarray(Wk.T).astype(np.float16),
        "wvt": np.ascontiguousarray(Wv.T).astype(np.float16),
        "wot": np.ascontiguousarray(Wo.T).astype(np.float16),
        "c0th": c0th, "c0tl": c0tl, "c20": c20,
        "bq": bq, "bk": bk, "outb": outb,
    }
    cat = {"xh": xh, "xl": xl}
    for k_, v in rep.items():
        cat[k_] = np.concatenate([v] * NCORES, axis=0)
    args = [cat[k_] for k_ in in_names]
    outs = fn(*args, *mkzeros())
    out = np.asarray(outs[0]).astype(np.float32)
    return out.reshape(1, N, E)
